# revision 1
# baseline (speedup 1.0000x reference)
"""TRN2 Bass kernel for nn_MultiHeadSelfAttention_15822659518596.

Key algebraic fact: in the reference, softmax and V are dead code — the
output is

    out[b,i,:] = (scores[b,i].reshape(S*H)) @ W_fc.T + b_fc
    scores[b,i,j,n] = (q[b,i,n,:] . k[b,j,n,:]) / 8

which collapses into dense GEMMs without materializing the (B,S,S,H)
score tensor:

    Kf_b = x_b @ Wk.T + bk                  (S, D)   [c = n*64+kk head-major]
    M_b[c,o] = sum_j Kf_b[j,c] * Wfc[o, j*8+n(c)] / 8        (D, D)
    qT_b = Wq @ x_b.T + bq                  (D, S)
    out_b = qT_b.T @ M_b + b_fc             (S, D)

Sharding: 8 cores = (4 batches) x (2 halves of the fc output dim o).
Each core computes outT[o_half, S] for its (b, h) — no collectives.
W_fc is pre-scaled by 1/8 on the host; the b_qkv k-bias enters M exactly
via a rank-1 matmul with host-precomputed per-head column sums.

All matmuls run as float32r (fp32 storage, ~1e-4 rel.err, 4x fp32 speed).
"""

import ml_dtypes
import numpy as np

import concourse.bass as bass
import concourse.tile as tile
from concourse import mybir, bacc
from concourse.bass_utils import run_bass_kernel_spmd
from concourse.tile import add_dep_helper as _adh
USE_DEP_CHAINS = False
def add_dep_helper(*a, **k):
    if USE_DEP_CHAINS:
        _adh(*a, **k)

B, S, D, H = 4, 2048, 512, 8
DK = D // H            # 64
OH = D // 2            # 256, per-core o-half
NC = 8                 # cores
F32 = mybir.dt.float32
F32R = mybir.dt.float32r
BF16 = mybir.dt.bfloat16
COPY = mybir.ActivationFunctionType.Identity

_CACHE = {}


def _build_program():
    """One SPMD Bass program; per-core tensors differ only in data."""
    nc = bacc.Bacc("TRN2", target_bir_lowering=False, debug=False, num_devices=NC)

    xT = nc.dram_tensor("xT", [D, S], BF16, kind="ExternalInput")          # x_b.T
    wqT = nc.dram_tensor("wqT", [D, D], BF16, kind="ExternalInput")        # [d, c]
    wkT = nc.dram_tensor("wkT", [D, D], BF16, kind="ExternalInput")        # [d, c]
    wfc = nc.dram_tensor("wfc", [H, 128, 16 * OH], BF16, kind="ExternalInput")
    colsum = nc.dram_tensor("colsum", [1, H * OH], BF16, kind="ExternalInput")
    bkrow = nc.dram_tensor("bkrow", [1, D], BF16, kind="ExternalInput")
    bqt = nc.dram_tensor("bqt", [128, 4], F32, kind="ExternalInput")      # bq.reshape(4,128).T
    bfct = nc.dram_tensor("bfct", [128, 2], F32, kind="ExternalInput")    # bfc_half.reshape(2,128).T
    outT = nc.dram_tensor("outT", [OH, S], F32, kind="ExternalOutput")

    with tile.TileContext(nc) as tc:
        with tc.tile_pool(name="xt", bufs=4) as p_xt, \
             tc.tile_pool(name="wq", bufs=4) as p_wq, \
             tc.tile_pool(name="wk", bufs=4) as p_wk, \
             tc.tile_pool(name="kf", bufs=16) as p_kf, \
             tc.tile_pool(name="qt", bufs=4) as p_qt, \
             tc.tile_pool(name="m", bufs=4) as p_m, \
             tc.tile_pool(name="wf", bufs=8) as p_wf, \
             tc.tile_pool(name="ob", bufs=3) as p_ob, \
             tc.tile_pool(name="bias", bufs=1) as p_bias, \
             tc.tile_pool(name="ps1", bufs=2, space="PSUM") as ps1, \
             tc.tile_pool(name="ps2", bufs=2, space="PSUM") as ps2, \
             tc.tile_pool(name="ps3", bufs=2, space="PSUM") as ps3, \
             tc.tile_pool(name="ps4", bufs=2, space="PSUM") as ps4:

            # ---- input DMAs. x and Wk first (chained so the x stream is
            # sequential and stage 1 starts within ~4us); Wq and the big wfc
            # stream are gated behind the last x chunk so they don't steal
            # HBM bandwidth from the critical path ----
            xts, wqs, wks = [], [], []
            x_fh_last = None
            for di in range(4):
                t_x = p_xt.tile([128, S], BF16, tag="xt")
                # two j-half DMAs: byte-range dep tracking lets the first
                # 8 stage-1 groups start before the second halves land
                x_fh_last = nc.sync.dma_start(t_x[:, :S // 2],
                                              xT[di * 128:(di + 1) * 128, :S // 2])
                nc.sync.dma_start(t_x[:, S // 2:],
                                  xT[di * 128:(di + 1) * 128, S // 2:])
                xts.append(t_x)
                t_k = p_wk.tile([128, D], BF16, tag="wk")
                nc.sync.dma_start(t_k[:], wkT[di * 128:(di + 1) * 128, :])
                wks.append(t_k)
            for di in range(4):
                t_q = p_wq.tile([128, D], BF16, tag="wq")
                nc.sync.dma_start(t_q[:], wqT[di * 128:(di + 1) * 128, :])
                wqs.append(t_q)
            t_bq = p_bias.tile([128, 4], F32, tag="bq")
            nc.sync.dma_start(t_bq[:], bqt[:])
            t_bfc = p_bias.tile([128, 2], F32, tag="bfc")
            nc.sync.dma_start(t_bfc[:], bfct[:])
            t_bk = p_bias.tile([1, D], BF16, tag="bk")
            nc.sync.dma_start(t_bk[:], bkrow[:])
            t_cs = p_bias.tile([1, H * OH], BF16, tag="cs")
            nc.sync.dma_start(t_cs[:], colsum[:])

            # ---- stage 1: Kf[j, c] (16 j-tiles), Kf = x @ Wk.T ----
            kfs = []
            for jt in range(16):
                pk = ps1.tile([128, D], F32)
                for di in range(4):
                    nc.tensor.matmul(
                        pk[:], xts[di][:, jt * 128:(jt + 1) * 128], wks[di][:],
                        start=(di == 0), stop=(di == 3))
                t_kf = p_kf.tile([128, D], BF16, tag="kf")
                nc.vector.tensor_copy(t_kf[:], pk[:])
                kfs.append(t_kf)

            # ---- stage 2: M[c, o] per head pair u. bf16 matmuls support PE
            # column-group tiling, so head 2u accumulates into psum[0:64]
            # (col group 0) while head 2u+1 goes to psum[64:128] (col group
            # 64) — concurrent in the array, one (128, OH) psum bank.
            # wfc head DMAs are chained behind the x stream and each other so
            # arrivals match consumption order ----
            ms = []
            for u in range(4):
                n0, n1 = 2 * u, 2 * u + 1
                t_w0 = p_wf.tile([128, 16 * OH], BF16, tag="wf")
                nc.sync.dma_start(t_w0[:, :8 * OH], wfc[n0][:, :8 * OH])
                nc.sync.dma_start(t_w0[:, 8 * OH:], wfc[n0][:, 8 * OH:])
                t_w1 = p_wf.tile([128, 16 * OH], BF16, tag="wf")
                nc.sync.dma_start(t_w1[:, :8 * OH], wfc[n1][:, :8 * OH])
                nc.sync.dma_start(t_w1[:, 8 * OH:], wfc[n1][:, 8 * OH:])
                pm = ps2.tile([128, OH], F32)
                # Zero the bank with DVE and run every matmul start=False:
                # per-element has_written semantics then make any schedule
                # order of the two disjoint col-group chains correct (a
                # start=True matmul would clear the WHOLE bank and race the
                # other chain, which Tile cannot see as a WAW hazard).
                nc.vector.memset(pm[:], 0.0)
                for jt in range(16):
                    nc.tensor.matmul(
                        pm[0:64, :], kfs[jt][:, n0 * 64:(n0 + 1) * 64],
                        t_w0[:, jt * OH:(jt + 1) * OH],
                        start=False, stop=False, tile_position=(0, 0),
                        skip_group_check=True)
                    nc.tensor.matmul(
                        pm[64:128, :], kfs[jt][:, n1 * 64:(n1 + 1) * 64],
                        t_w1[:, jt * OH:(jt + 1) * OH],
                        start=False, stop=False, tile_position=(0, 64),
                        skip_group_check=True)
                # exact b_qkv k-bias: M += bk[c] (x) colsum_n
                nc.tensor.matmul(
                    pm[0:64, :], t_bk[0:1, n0 * 64:(n0 + 1) * 64],
                    t_cs[0:1, n0 * OH:(n0 + 1) * OH],
                    start=False, stop=False, tile_position=(0, 0),
                    skip_group_check=True)
                nc.tensor.matmul(
                    pm[64:128, :], t_bk[0:1, n1 * 64:(n1 + 1) * 64],
                    t_cs[0:1, n1 * OH:(n1 + 1) * OH],
                    start=False, stop=True, tile_position=(0, 64),
                    skip_group_check=True)
                t_m = p_m.tile([128, OH], F32R, tag="m")
                nc.vector.tensor_copy(t_m[:], pm[:])
                ms.append(t_m)

            # ---- stages 3+4 fused per i-chunk: compute the four qT
            # c-tiles for chunk ic, then immediately contract with M and
            # stream the output chunk out. Keeps stage-4 + out-DMA off the
            # kernel tail ----
            qts = []
            for ct in range(4):
                t_qt = p_qt.tile([128, S], F32R, tag="qt")
                qts.append(t_qt)
            for ic in range(4):
                for ct in range(4):
                    pq = ps3.tile([128, 512], F32)
                    for di in range(4):
                        nc.tensor.matmul(
                            pq[:], wqs[di][:, ct * 128:(ct + 1) * 128],
                            xts[di][:, ic * 512:(ic + 1) * 512],
                            start=(di == 0), stop=(di == 3))
                    nc.scalar.activation(
                        qts[ct][:, ic * 512:(ic + 1) * 512], pq[:], COPY,
                        bias=t_bq[:, ct:ct + 1])
                for ot in range(2):
                    po = ps4.tile([128, 512], F32)
                    for u in range(4):
                        nc.tensor.matmul(
                            po[:], ms[u][:, ot * 128:(ot + 1) * 128],
                            qts[u][:, ic * 512:(ic + 1) * 512],
                            start=(u == 0), stop=(u == 3))
                    t_o = p_ob.tile([128, 512], F32, tag="ob")
                    nc.vector.tensor_scalar_add(t_o[:], po[:],
                                                t_bfc[:, ot:ot + 1])
                    nc.sync.dma_start(
                        outT[ot * 128:(ot + 1) * 128, ic * 512:(ic + 1) * 512],
                        t_o[:])
    nc.compile()
    return nc


def _prep_inputs(x, W_qkv, b_qkv, W_fc, b_fc):
    """Host-side sharding/layout prep. O(bytes) only — no GEMM work."""
    x = np.ascontiguousarray(x, dtype=np.float32)
    W_qkv = np.asarray(W_qkv, dtype=np.float32)
    b_qkv = np.asarray(b_qkv, dtype=np.float32)
    W_fc = np.asarray(W_fc, dtype=np.float32)
    b_fc = np.asarray(b_fc, dtype=np.float32)

    wq = W_qkv.reshape(H, 3, DK, D)  # [n, {q,k,v}, kk, d]
    wqT = np.ascontiguousarray(wq[:, 0].reshape(D, D).T).astype(ml_dtypes.bfloat16)  # [d, c]
    wkT = np.ascontiguousarray(wq[:, 1].reshape(D, D).T).astype(ml_dtypes.bfloat16)
    bq = b_qkv.reshape(H, 3, DK)
    bq_c = np.ascontiguousarray(bq[:, 0].reshape(D))      # c-order
    bk_c = np.ascontiguousarray(bq[:, 1].reshape(D))
    bqt = np.ascontiguousarray(bq_c.reshape(4, 128).T)    # (128, 4)
    bkrow = bk_c.reshape(1, D).astype(ml_dtypes.bfloat16)

    Wfc_s = W_fc * (1.0 / 8.0)
    # per o-half h: [n, jj, t, o] layout, plus per-head column sums
    wfc_h, cs_h, bfct_h = [], [], []
    for h in range(2):
        A = Wfc_s[h * OH:(h + 1) * OH, :]                  # (256, 16384)
        arr = np.ascontiguousarray(A.T).reshape(S, H, OH).transpose(1, 0, 2)  # [n,j,o]
        cs = np.ascontiguousarray(arr.sum(axis=1)).reshape(1, H * OH)
        arr2 = np.ascontiguousarray(
            arr.reshape(H, 16, 128, OH).transpose(0, 2, 1, 3)  # [n, jj, t, o]
        ).reshape(H, 128, 16 * OH).astype(ml_dtypes.bfloat16)
        wfc_h.append(arr2)
        cs_h.append(cs.astype(ml_dtypes.bfloat16))
        bfct_h.append(np.ascontiguousarray(
            b_fc[h * OH:(h + 1) * OH].reshape(2, 128).T))

    xT_b = [np.ascontiguousarray(x[b].T).astype(ml_dtypes.bfloat16) for b in range(B)]

    in_maps = []
    for c in range(NC):
        b, h = c // 2, c % 2
        in_maps.append({
            "xT": xT_b[b],
            "wqT": wqT,
            "wkT": wkT,
            "wfc": wfc_h[h],
            "colsum": cs_h[h],
            "bkrow": bkrow,
            "bqt": bqt,
            "bfct": bfct_h[h],
        })
    return in_maps


def _run(in_maps, trace=False, **kw):
    if "nc" not in _CACHE:
        _CACHE["nc"] = _build_program()
    return run_bass_kernel_spmd(
        _CACHE["nc"], in_maps, core_ids=list(range(NC)), trace=trace, **kw)


def _assemble(results):
    out = np.empty((B, S, D), dtype=np.float32)
    for c in range(NC):
        b, h = c // 2, c % 2
        out[b, :, h * OH:(h + 1) * OH] = results[c]["outT"].T
    return out


def kernel(x, W_qkv, b_qkv, W_fc, b_fc):
    in_maps = _prep_inputs(x, W_qkv, b_qkv, W_fc, b_fc)
    res = _run(in_maps, trace=False)
    return _assemble(res.results)


def kernel_traced(x, W_qkv, b_qkv, W_fc, b_fc):
    """Like kernel() but returns (out, BassKernelResults) with NTFF trace."""
    import os
    os.environ.setdefault("BASS_PERFETTO_PROFILE_ALL_CORES", "1")
    _install_ntff_hook_shim()
    in_maps = _prep_inputs(x, W_qkv, b_qkv, W_fc, b_fc)
    res = _run(in_maps, trace=True)
    return _assemble(res.results), res


def _install_ntff_hook_shim():
    """The agent image's antenv lacks axon_hooks; provide it so
    run_bass_kernel_spmd(trace=True) can reach the NTFF profiler."""
    import sys, types
    if "antenv.axon_hooks" in sys.modules:
        return
    try:
        from trn_agent_boot.trn_boot import _ntff_profile_via_ctypes
    except ImportError:
        return
    mod = types.ModuleType("antenv.axon_hooks")
    _hook = [None]
    mod.set_axon_ntff_profile_hook = lambda h: _hook.__setitem__(0, h)
    mod.get_axon_ntff_profile_hook = lambda: _hook[0]
    import antenv
    sys.modules["antenv.axon_hooks"] = mod
    antenv.axon_hooks = mod
    so = "/opt/axon/libaxon_pjrt.so"
    try:
        hook = _ntff_profile_via_ctypes(so)
    except OSError:
        hook = None
    mod.set_axon_ntff_profile_hook(hook)



# revision 4
# speedup vs baseline: 1.3050x; 1.3050x over previous
"""TRN2 Bass kernel for nn_MultiHeadSelfAttention_15822659518596.

Softmax and V are dead code in the reference; the output collapses to

    out_b = q_b @ M_b + b_fc,   q_b = x_b @ Wq.T + bq            (S, D)
    M_b[c,o] = sum_j Kf_b[j,c] * Wfc[o, j*8+n(c)] / 8            (D, D)
    Kf_b = x_b @ Wk.T + bk                                       (S, D)

Sharding: 8 cores = (4 batches) x (2 c-halves).  Core (b, h) owns heads
4h..4h+3 (c-half), computes Kf/qT for those c columns, builds the full
M[c-half, :] rows, and emits a PARTIAL output (contraction over its
c-half).  The two partials per batch are summed on the host (O(bytes)).
No device collectives; no duplicated matmul work across cores:
65,536 PE cycles/core (the 8-way-optimal count).

Precision: Wfc is shipped as fp8 e3m4 scaled by 512 (values sit in the
e3m4 normal range); the inverse 1/512 is folded into Wq/bq on the host,
so out = (q/512) @ (512*M) needs no on-chip rescaling.  Everything else
runs bf16 with f32 psum.  Measured end-to-end frobenius rel err ~1.4e-2
(budget 2e-2).
"""

import ml_dtypes
import numpy as np

import concourse.bass as bass
import concourse.tile as tile
from concourse import mybir, bacc
from concourse.bass_utils import run_bass_kernel_spmd

B, S, D, H = 4, 2048, 512, 8
DK = D // H            # 64
CH = D // 2            # 256 c-columns per core (4 heads)
NC = 8
F32 = mybir.dt.float32
BF16 = mybir.dt.bfloat16
FP8E3 = mybir.dt.float8e3
COPY = mybir.ActivationFunctionType.Identity
WFC_SCALE = 512.0

_CACHE = {}


def _build_program():
    """One SPMD Bass program; per-core tensors differ only in data."""
    nc = bacc.Bacc("TRN2", target_bir_lowering=False, debug=False, num_devices=NC)

    xT = nc.dram_tensor("xT", [D, S], BF16, kind="ExternalInput")       # x_b.T
    # packed per d-tile: cols [dt*512, dt*512+256) = (Wq/512).T slice,
    # [dt*512+256, dt*512+512) = Wk.T slice -- both restricted to c-half
    wqk = nc.dram_tensor("wqk", [128, 2048], BF16, kind="ExternalInput")
    # per local head: [j-part 128, jt 16 x o 512], e3m4, scaled by 512/8
    wfc8 = nc.dram_tensor("wfc8", [4, 128, 16 * D], FP8E3, kind="ExternalInput")
    bqt = nc.dram_tensor("bqt", [128, 2], F32, kind="ExternalInput")    # (bq/512) cols
    bkrow = nc.dram_tensor("bkrow", [1, CH], BF16, kind="ExternalInput")
    colsum = nc.dram_tensor("colsum", [1, 4 * D], BF16, kind="ExternalInput")
    outP = nc.dram_tensor("outP", [D, S], BF16, kind="ExternalOutput")  # partial out.T

    with tile.TileContext(nc) as tc:
        with tc.tile_pool(name="xt", bufs=4) as p_xt, \
             tc.tile_pool(name="w", bufs=1) as p_w, \
             tc.tile_pool(name="kf", bufs=16) as p_kf, \
             tc.tile_pool(name="qt", bufs=2) as p_qt, \
             tc.tile_pool(name="m", bufs=2) as p_m, \
             tc.tile_pool(name="wf", bufs=4) as p_wf, \
             tc.tile_pool(name="ob", bufs=3) as p_ob, \
             tc.tile_pool(name="bias", bufs=1) as p_bias, \
             tc.tile_pool(name="ps1", bufs=2, space="PSUM") as ps1, \
             tc.tile_pool(name="ps2", bufs=2, space="PSUM") as ps2, \
             tc.tile_pool(name="ps3", bufs=2, space="PSUM") as ps3, \
             tc.tile_pool(name="ps4", bufs=2, space="PSUM") as ps4:

            # ---- input DMAs, in stream-priority order: biases, weights,
            # x (j-half-major so stage 1/3 start early), then the big wfc
            # stream whose tail gates the kernel ----
            t_bq = p_bias.tile([128, 2], F32, tag="bq")
            nc.sync.dma_start(t_bq[:], bqt[:])
            t_bk = p_bias.tile([1, CH], BF16, tag="bk")
            nc.sync.dma_start(t_bk[:], bkrow[:])
            t_cs = p_bias.tile([1, 4 * D], BF16, tag="cs")
            nc.sync.dma_start(t_cs[:], colsum[:])
            t_w = p_w.tile([128, 2048], BF16, tag="w")
            nc.sync.dma_start(t_w[:], wqk[:])

            xts = []
            for di in range(4):
                xts.append(p_xt.tile([128, S], BF16, tag="xt", name=f"t_x{di}"))
            for jh in range(2):
                for di in range(4):
                    nc.sync.dma_start(
                        xts[di][:, jh * 1024:(jh + 1) * 1024],
                        xT[di * 128:(di + 1) * 128, jh * 1024:(jh + 1) * 1024])

            wfs = []
            for ln in range(4):
                t_wf = p_wf.tile([128, 16 * D], FP8E3, tag="wf")
                nc.sync.dma_start(t_wf[:, :8 * D], wfc8[ln][:, :8 * D])
                nc.sync.dma_start(t_wf[:, 8 * D:], wfc8[ln][:, 8 * D:])
                wfs.append(t_wf)

            # ---- stage 1: Kf[j, c-half] (16 j-tiles) ----
            kfs = []
            for jt in range(16):
                pk = ps1.tile([128, CH], F32)
                for di in range(4):
                    nc.tensor.matmul(
                        pk[:], xts[di][:, jt * 128:(jt + 1) * 128],
                        t_w[:, di * 512 + 256:(di + 1) * 512],
                        start=(di == 0), stop=(di == 3))
                t_kf = p_kf.tile([128, CH], BF16, tag="kf")
                nc.vector.tensor_copy(t_kf[:], pk[:])
                kfs.append(t_kf)

            # ---- stage 3: qT[c-half, i] scaled by 1/512 ----
            qts = [p_qt.tile([128, S], BF16, tag="qt", name=f"t_q{ct}")
                   for ct in range(2)]
            for ic in range(4):
                for ct in range(2):
                    pq = ps3.tile([128, 512], F32)
                    for di in range(4):
                        nc.tensor.matmul(
                            pq[:],
                            t_w[:, di * 512 + ct * 128:di * 512 + (ct + 1) * 128],
                            xts[di][:, ic * 512:(ic + 1) * 512],
                            start=(di == 0), stop=(di == 3))
                    nc.scalar.activation(
                        qts[ct][:, ic * 512:(ic + 1) * 512], pq[:], COPY,
                        bias=t_bq[:, ct:ct + 1])

            # ---- stage 2: M rows for this c-half, one psum per head pair.
            # bf16/fp8 matmuls support PE column-group tiling: head 2u in
            # psum[0:64] (col group 0), head 2u+1 in psum[64:128] (group 64),
            # concurrent in the array.  memset + start=False everywhere so
            # the two disjoint chains can't WAW-race on bank clears. ----
            ms = []
            for u in range(2):
                n0, n1 = 2 * u, 2 * u + 1
                pm = ps2.tile([128, D], F32)
                nc.vector.memset(pm[:], 0.0)
                for jt in range(16):
                    nc.tensor.matmul(
                        pm[0:64, :], kfs[jt][:, n0 * 64:(n0 + 1) * 64],
                        wfs[n0][:, jt * D:(jt + 1) * D],
                        start=False, stop=False, tile_position=(0, 0),
                        skip_group_check=True)
                    nc.tensor.matmul(
                        pm[64:128, :], kfs[jt][:, n1 * 64:(n1 + 1) * 64],
                        wfs[n1][:, jt * D:(jt + 1) * D],
                        start=False, stop=False, tile_position=(0, 64),
                        skip_group_check=True)
                # exact b_qkv k-bias: M += bk[c] (x) colsum_n[o]
                nc.tensor.matmul(
                    pm[0:64, :], t_bk[0:1, n0 * 64:(n0 + 1) * 64],
                    t_cs[0:1, n0 * D:(n0 + 1) * D],
                    start=False, stop=False, tile_position=(0, 0),
                    skip_group_check=True)
                nc.tensor.matmul(
                    pm[64:128, :], t_bk[0:1, n1 * 64:(n1 + 1) * 64],
                    t_cs[0:1, n1 * D:(n1 + 1) * D],
                    start=False, stop=True, tile_position=(0, 64),
                    skip_group_check=True)
                t_m = p_m.tile([128, D], BF16, tag="m")
                nc.vector.tensor_copy(t_m[:], pm[:])
                ms.append(t_m)

            # ---- stage 4: partial outT[o, i] = sum_{c-half} M.T-contract ----
            for ot in range(4):
                t_o = p_ob.tile([128, S], BF16, tag="ob")
                for ic in range(4):
                    po = ps4.tile([128, 512], F32)
                    for u in range(2):
                        nc.tensor.matmul(
                            po[:], ms[u][:, ot * 128:(ot + 1) * 128],
                            qts[u][:, ic * 512:(ic + 1) * 512],
                            start=(u == 0), stop=(u == 1))
                    nc.vector.tensor_copy(t_o[:, ic * 512:(ic + 1) * 512], po[:])
                nc.sync.dma_start(outP[ot * 128:(ot + 1) * 128, :], t_o[:])
    nc.compile()
    return nc


def _prep_inputs(x, W_qkv, b_qkv, W_fc, b_fc):
    """Host-side sharding/layout prep. O(bytes) only -- no GEMM work."""
    x = np.ascontiguousarray(x, dtype=np.float32)
    W_qkv = np.asarray(W_qkv, dtype=np.float32)
    b_qkv = np.asarray(b_qkv, dtype=np.float32)
    W_fc = np.asarray(W_fc, dtype=np.float32)

    wq3 = W_qkv.reshape(H, 3, DK, D)          # [n, {q,k,v}, kk, d]
    wq = wq3[:, 0].reshape(D, D)              # [c, d], c = n*64+kk
    wk = wq3[:, 1].reshape(D, D)
    bq3 = b_qkv.reshape(H, 3, DK)
    bq_c = np.ascontiguousarray(bq3[:, 0].reshape(D))
    bk_c = np.ascontiguousarray(bq3[:, 1].reshape(D))

    # [j, n, o] view of Wfc scaled by 512/8; e3m4 wants values ~O(1)
    G = np.ascontiguousarray((W_fc * (WFC_SCALE / 8.0)).T).reshape(S, H, D)

    in_maps = [dict() for _ in range(NC)]
    for b in range(B):
        xT_b = np.ascontiguousarray(x[b].T).astype(ml_dtypes.bfloat16)
        in_maps[2 * b]["xT"] = xT_b
        in_maps[2 * b + 1]["xT"] = xT_b
    for h in range(2):
        cs, ce = h * CH, (h + 1) * CH
        wqT = np.ascontiguousarray((wq[cs:ce, :] / WFC_SCALE).T)  # [d, 256]
        wkT = np.ascontiguousarray(wk[cs:ce, :].T)
        wpack = np.empty((128, 2048), np.float32)
        for dt in range(4):
            wpack[:, dt * 512:dt * 512 + 256] = wqT[dt * 128:(dt + 1) * 128, :]
            wpack[:, dt * 512 + 256:(dt + 1) * 512] = wkT[dt * 128:(dt + 1) * 128, :]
        wpack = wpack.astype(ml_dtypes.bfloat16)
        bqt = np.ascontiguousarray(
            (bq_c[cs:ce] / WFC_SCALE).reshape(2, 128).T).astype(np.float32)
        bkrow = bk_c[cs:ce].reshape(1, CH).astype(ml_dtypes.bfloat16)

        wfc8 = np.empty((4, 128, 16 * D), ml_dtypes.float8_e3m4)
        csum = np.empty((1, 4 * D), np.float32)
        for ln in range(4):
            n = 4 * h + ln
            head = G[:, n, :]                 # [2048 j, 512 o]
            t8 = np.ascontiguousarray(
                head.reshape(16, 128, D).transpose(1, 0, 2).reshape(128, 16 * D)
            ).astype(ml_dtypes.float8_e3m4)
            wfc8[ln] = t8
            # colsum over the QUANTIZED values so the bk rank-1 term is exact
            csum[0, ln * D:(ln + 1) * D] = (
                t8.astype(np.float32).reshape(128, 16, D).sum(axis=(0, 1)))
        csum_b = csum.astype(ml_dtypes.bfloat16)

        for b in range(B):
            in_maps[2 * b + h].update({
                "wqk": wpack, "wfc8": wfc8, "bqt": bqt,
                "bkrow": bkrow, "colsum": csum_b,
            })
    return in_maps, np.asarray(b_fc, dtype=np.float32)


def _run(in_maps, trace=False, **kw):
    if "nc" not in _CACHE:
        _CACHE["nc"] = _build_program()
    return run_bass_kernel_spmd(
        _CACHE["nc"], in_maps, core_ids=list(range(NC)), trace=trace, **kw)


def _assemble(results, b_fc):
    out = np.empty((B, S, D), dtype=np.float32)
    for b in range(B):
        p = (results[2 * b]["outP"].astype(np.float32)
             + results[2 * b + 1]["outP"].astype(np.float32))
        out[b] = p.T + b_fc
    return out


def kernel(x, W_qkv, b_qkv, W_fc, b_fc):
    in_maps, bfc = _prep_inputs(x, W_qkv, b_qkv, W_fc, b_fc)
    res = _run(in_maps, trace=False)
    return _assemble(res.results, bfc)


def kernel_traced(x, W_qkv, b_qkv, W_fc, b_fc):
    """Like kernel() but returns (out, BassKernelResults) with NTFF trace."""
    import os
    os.environ.setdefault("BASS_PERFETTO_PROFILE_ALL_CORES", "1")
    _install_ntff_hook_shim()
    in_maps, bfc = _prep_inputs(x, W_qkv, b_qkv, W_fc, b_fc)
    res = _run(in_maps, trace=True)
    return _assemble(res.results, bfc), res


def _install_ntff_hook_shim():
    """The agent image's antenv lacks axon_hooks; provide it so
    run_bass_kernel_spmd(trace=True) can reach the NTFF profiler."""
    import sys, types
    if "antenv.axon_hooks" in sys.modules:
        return
    try:
        from trn_agent_boot.trn_boot import _ntff_profile_via_ctypes
    except ImportError:
        return
    mod = types.ModuleType("antenv.axon_hooks")
    _hook = [None]
    mod.set_axon_ntff_profile_hook = lambda h: _hook.__setitem__(0, h)
    mod.get_axon_ntff_profile_hook = lambda: _hook[0]
    import antenv
    sys.modules["antenv.axon_hooks"] = mod
    antenv.axon_hooks = mod
    so = "/opt/axon/libaxon_pjrt.so"
    try:
        hook = _ntff_profile_via_ctypes(so)
    except OSError:
        hook = None
    mod.set_axon_ntff_profile_hook(hook)
